# revision 18
# baseline (speedup 1.0000x reference)
"""DecoderRNN Trainium2 kernel (data-parallel over 8 NeuronCores).

Computation (see reference): 2 branches x 10 steps x 2 LSTM cells, strictly
sequential (h, c, prev carry through all 40 cell evals), batch 4096, hidden
1024, input 2048+128+128.

Structure per core (batch shard BL=512), transposed layout (partition dim =
feature, free dim = batch) so the recurrence needs no transposes:

  - Zpre[b] = (64*W_z) @ z_b.T  (4096, BL) precomputed once per branch on
    device, kept RESIDENT IN SBUF (no DRAM round-trip in the cell loop).
  - The recurrence h @ W_hh.T runs in fp8e4 with MatmulPerfMode.DoubleRow
    (2 K-tiles per instruction). All step-varying weights are scaled x64 so
    fp8 stays in the normal range; the gate activation descales via its
    scale=1/64 immediate. h is stored fp8 directly by the DVE.
  - action_emb contribution -> 16-row table T_act = 64 * action_emb @
    W_act.T, applied per step as a K=16 one-hot matmul (bf16, 4 concurrent
    32-row PE strips) accumulated into the same PSUM group.
  - Per gate: DVE adds Zpre into PSUM in place, ScalarE applies
    sigmoid/tanh (scale=1/64, per-partition bias) writing bf16; c kept in
    bf16; ci/cf products on GPSIMD, c/h updates on DVE (h written as fp8
    for the next matmul).
  - tiny heads (4/11 logits) matmul'd per step in fp8 DoubleRow; log-softmax
    DEFERRED: logits stored packed (4 steps per 32-partition group), one
    Exp/Ln table switch at the end, partition-group reduction via fp32
    matmul with a 0/1 matrix.
  - all 40 one-hot tiles are prefetched into SBUF before the cell loop.

Host side does only index/layout prep: weight transposes + dtype casts,
one-hot encodings of the (input) action sequences, and the final
(2,10,BL) -> (BL,2,10) transpose of the gathered outputs.
"""

import numpy as np
import ml_dtypes

import concourse.bass as bass
import concourse.bacc as bacc
import concourse.mybir as mybir
import concourse.tile as tile
from concourse.bass_utils import run_bass_kernel_spmd

f32 = mybir.dt.float32
bf16 = mybir.dt.bfloat16
f8 = mybir.dt.float8e4
AF = mybir.ActivationFunctionType
DR = mybir.MatmulPerfMode.DoubleRow
DRSW = mybir.MatmulPerfMode.DoubleRowSwInterleave

B, ENC, DEC, EMB, S = 4096, 2048, 1024, 128, 10
NT, NM = 4, 11
VOCAB = NT + NM + 1  # 16, start token 15
NCORES = 8
BL = B // NCORES     # 512 batch rows per core
KZ = ENC // 128      # 16 K-tiles for z projection
KH = DEC // 128      # 8 K-tiles for h projection
KP = KH // 2         # 4 K-pair tiles for fp8 DoubleRow
MT = 4 * DEC // 128  # 32 M-tiles over the gate dim
NCALL = 2 * S * 2    # 40 sequential LSTM cell evals
NBLK = 2 * S         # 20 (branch, step) head blocks
NQ = (NBLK + 3) // 4  # 5 column groups of 4 blocks in packed logit layout

WSCALE = 64.0        # PSUM domain scale (zpre/tact x64; descaled by ACT)
HSCALE = 8.0         # h stored as 8*h in fp8 (dodges e4m3 denormals);
                     # recurrence/head weights carry WSCALE/HSCALE = x8

# matmul variant for the recurrence + heads: "dr" (fp8 DoubleRow),
# "drsw" (fp8 DoubleRowSwInterleave), "bf16"
MM = "drsw"

_CACHE = {}


def _build_program(trace_friendly=False, ncall=NCALL, do_p1=True, do_epi=True,
                   mm=None):
    mm = MM if mm is None else mm
    wdt = bf16 if mm == "bf16" else f8
    nc = bacc.Bacc("TRN2")

    zT_d = nc.dram_tensor("zT", (128, 2, KZ, BL), bf16, kind="ExternalInput")
    wzT_d = nc.dram_tensor("wzT", (128, KZ, 4 * DEC), bf16, kind="ExternalInput")
    if mm == "drsw":
        whhT_d = nc.dram_tensor("whhT", (128, KP, 2 * 4 * DEC), f8,
                                kind="ExternalInput")
    else:
        whhT_d = nc.dram_tensor("whhT", (128, KH, 4 * DEC), wdt,
                                kind="ExternalInput")
    tact_d = nc.dram_tensor("tact", (128, 4 * DEC), bf16, kind="ExternalInput")
    ball_d = nc.dram_tensor("ball", (128, MT), f32, kind="ExternalInput")
    oneh_d = nc.dram_tensor("oneh", (128, NCALL, BL), bf16, kind="ExternalInput")
    oselt_d = nc.dram_tensor("oselt", (128, NQ * BL), f32, kind="ExternalInput")
    oselm_d = nc.dram_tensor("oselm", (128, NQ * BL), f32, kind="ExternalInput")
    nhp = 16 if mm != "bf16" else NT  # DoubleRow needs 16B k-pair stride
    nmp = 16 if mm != "bf16" else NM
    wtT_d = nc.dram_tensor("wtT", (128, KH, nhp), wdt, kind="ExternalInput")
    wmT_d = nc.dram_tensor("wmT", (128, KH, nmp), wdt, kind="ExternalInput")
    bt_d = nc.dram_tensor("bt", (NT, 1), f32, kind="ExternalInput")
    bm_d = nc.dram_tensor("bm", (NM, 1), f32, kind="ExternalInput")
    redt_d = nc.dram_tensor("redt", (128, 4), f32, kind="ExternalInput")
    redm_d = nc.dram_tensor("redm", (128, 4), f32, kind="ExternalInput")

    tout_d = nc.dram_tensor("tout", (2, S, BL), f32, kind="ExternalOutput")
    mout_d = nc.dram_tensor("mout", (2, S, BL), f32, kind="ExternalOutput")

    inv = 1.0 / WSCALE if mm != "bf16" else 1.0

    with tile.TileContext(nc) as tc:
        with tc.tile_pool(name="res", bufs=1) as res, \
             tc.tile_pool(name="hbuf", bufs=2) as hbuf:

            if mm == "drsw":
                whh_s = res.tile([128, KP, 2 * 4 * DEC], f8)
            else:
                whh_s = res.tile([128, KH, 4 * DEC], wdt)
            nc.sync.dma_start(whh_s[:], whhT_d[:])
            tact_s = res.tile([128, 4 * DEC], bf16)
            nc.sync.dma_start(tact_s[:], tact_d[:])
            ball_s = res.tile([128, MT], f32)
            nc.sync.dma_start(ball_s[:], ball_d[:])
            wtT_s = res.tile([128, KH, nhp], wdt)
            nc.sync.dma_start(wtT_s[:], wtT_d[:])
            wmT_s = res.tile([128, KH, nmp], wdt)
            nc.sync.dma_start(wmT_s[:], wmT_d[:])
            bt_s = res.tile([NT, 1], f32)
            nc.sync.dma_start(bt_s[:], bt_d[:])
            bm_s = res.tile([NM, 1], f32)
            nc.sync.dma_start(bm_s[:], bm_d[:])

            # packed logit buffers: block (b*10+s) = 4q+r lives at
            # partitions 32r.. (4 rows for t, 11 for m), free cols q*BL..
            lt_s = res.tile([128, NQ * BL], f32)
            lm_s = res.tile([128, NQ * BL], f32)
            nc.vector.memset(lt_s[:], 0.0)
            nc.vector.memset(lm_s[:], 0.0)

            # persistent cell state (fp32) and ping-pong h (fp8/bf16)
            c_s = res.tile([128, KH, BL], f32)
            nc.vector.memset(c_s[:], 0.0)
            h_prev = hbuf.tile([128, KH, BL], wdt, tag="h")
            nc.vector.memset(h_prev[:], 0.0)

            # phase-1/2-lifetime buffers, freed before the epilogue:
            # SBUF-resident z projection [br, p, j, g, n] (x64-scaled bf16)
            # and all one-hot tiles up front (no per-cell DMA dependency)
            p12_cm = tc.tile_pool(name="p12", bufs=1)
            p12 = p12_cm.__enter__()
            zpre_s = p12.tile([128, 2, KH, 4, BL], bf16)
            oneh_s = p12.tile([128, NCALL, BL], bf16)
            csz = max(1, NCALL // 8)
            for c0 in range(0, NCALL, csz):
                c1 = min(c0 + csz, NCALL)
                nc.sync.dma_start(oneh_s[:, c0:c1, :], oneh_d[:, c0:c1, :])

            # ---------------- phase 1: Zpre = (64 Wz) @ z.T ---------------
            with tc.tile_pool(name="p1zt", bufs=6) as p1zt, \
                 tc.tile_pool(name="p1wz", bufs=6) as p1wz, \
                 tc.tile_pool(name="p1ps", bufs=8, space="PSUM") as p1ps:
                for mg in range(4 if do_p1 else 0):          # m-groups of 8
                    for br in range(2):
                        psums = []
                        for m8 in range(8):
                            psums.append(p1ps.tile([128, BL], f32, tag="zp_ps",
                                                   name="zp_ps"))
                        for k in range(KZ):
                            wz_k = p1wz.tile([128, 8 * 128], bf16, tag="wz")
                            nc.sync.dma_start(
                                wz_k[:], wzT_d[:, k, mg * 1024:(mg + 1) * 1024])
                            zt_k = p1zt.tile([128, BL], bf16, tag="zt")
                            nc.sync.dma_start(zt_k[:], zT_d[:, br, k, :])
                            for m8 in range(8):
                                nc.tensor.matmul(
                                    psums[m8][:],
                                    wz_k[:, m8 * 128:(m8 + 1) * 128],
                                    zt_k[:],
                                    start=(k == 0), stop=(k == KZ - 1))
                        for m8 in range(8):
                            m = mg * 8 + m8
                            nc.scalar.activation(
                                zpre_s[:, br, m % KH, m // KH, :],
                                psums[m8][:], AF.Identity)

            # ---------------- phase 2: 40 sequential LSTM cells -----------
            with tc.tile_pool(name="actp", bufs=6) as actp, \
                 tc.tile_pool(name="cmix", bufs=2) as cmix, \
                 tc.tile_pool(name="tcp", bufs=2) as tcp, \
                 tc.tile_pool(name="ps", bufs=8, space="PSUM") as ps:
                for li in range(ncall):
                    br = (li // (2 * S)) % 2
                    s = (li % (2 * S)) // 2
                    is_m = li % 2  # 0: transform head, 1: magnitude head
                    oh = oneh_s[:, li % NCALL, :]

                    h_new = hbuf.tile([128, KH, BL], wdt, tag="h")
                    for j in range(KH):  # hidden chunk
                        acts = []
                        pts = []
                        for g in range(4):
                            m = g * KH + j
                            pt = ps.tile([128, BL], f32, tag="ps", name="pt")
                            # 4 concurrent K=16 one-hot matmuls, one per gate
                            # bank, on disjoint 32-row strips of the PE array
                            nc.tensor.matmul(
                                pt[:],
                                tact_s[32 * g:32 * g + VOCAB,
                                       m * 128:(m + 1) * 128],
                                oh[32 * g:32 * g + VOCAB, :],
                                start=True, stop=False,
                                tile_position=(32 * g, 0))
                            pts.append(pt)
                        # k-major emission: all 4 gate groups advance
                        # together, so the stall on the latest h pair comes
                        # as late as possible in the PE queue
                        if mm == "bf16":
                            for k in range(KH):
                                for g in range(4):
                                    m = g * KH + j
                                    nc.tensor.matmul(
                                        pts[g][:],
                                        whh_s[:, k, m * 128:(m + 1) * 128],
                                        h_prev[:, k, :],
                                        start=False, stop=(k == KH - 1))
                        elif mm == "dr":
                            for k in range(0, KH, 2):
                                for g in range(4):
                                    m = g * KH + j
                                    nc.tensor.matmul(
                                        pts[g][:],
                                        whh_s[:, k:k + 2,
                                              m * 128:(m + 1) * 128],
                                        h_prev[:, k:k + 2, :],
                                        start=False, stop=(k == KH - 2),
                                        perf_mode=DR)
                        else:  # drsw
                            for kp in range(KP):
                                for g in range(4):
                                    m = g * KH + j
                                    wv = whh_s[:, kp, 2 * m * 128:
                                               2 * (m + 1) * 128].rearrange(
                                        "p (m2 two) -> p m2 two", two=2)
                                    nc.tensor.matmul(
                                        pts[g][:], wv,
                                        h_prev[:, 2 * kp:2 * kp + 2, :],
                                        start=False, stop=(kp == KP - 1),
                                        perf_mode=DRSW)
                        for g in range(4):
                            m = g * KH + j
                            # Zpre folded into PSUM in place, then activation
                            # descales (x64 domain) with per-partition bias
                            nc.vector.tensor_add(
                                pts[g][:], pts[g][:], zpre_s[:, br, j, g, :])
                            av = actp.tile([128, BL], bf16, tag="act")
                            fn = AF.Tanh if g == 2 else AF.Sigmoid
                            nc.scalar.activation(
                                av[:], pts[g][:], fn,
                                bias=ball_s[:, m:m + 1], scale=inv)
                            acts.append(av)
                        a_i, a_f, a_g, a_o = acts
                        cf = cmix.tile([128, BL], f32, tag="cf")
                        nc.gpsimd.tensor_mul(cf[:], a_f[:], c_s[:, j, :])
                        ci = cmix.tile([128, BL], f32, tag="ci")
                        nc.gpsimd.tensor_mul(ci[:], a_i[:], a_g[:])
                        nc.vector.tensor_add(c_s[:, j, :], cf[:], ci[:])
                        tch = tcp.tile([128, BL], bf16, tag="tc")
                        nc.scalar.activation(tch[:], c_s[:, j, :], AF.Tanh)
                        if mm == "bf16":
                            nc.vector.tensor_mul(
                                h_new[:, j, :], a_o[:], tch[:])
                        else:
                            # h stored as 8*h for fp8 range
                            nc.vector.scalar_tensor_tensor(
                                h_new[:, j, :], a_o[:], HSCALE, tch[:],
                                mybir.AluOpType.mult, mybir.AluOpType.mult)

                    # head on h_new
                    nh = NM if is_m else NT
                    wh = wmT_s if is_m else wtT_s
                    bh = bm_s if is_m else bt_s
                    ldst = lm_s if is_m else lt_s
                    hd = ps.tile([128, BL], f32, tag="ps")
                    if mm == "bf16":
                        for k in range(KH):
                            nc.tensor.matmul(
                                hd[0:nh, :], wh[:, k, :], h_new[:, k, :],
                                start=(k == 0), stop=(k == KH - 1))
                    else:
                        for k in range(0, KH, 2):
                            nc.tensor.matmul(
                                hd[0:16, :], wh[:, k:k + 2, :],
                                h_new[:, k:k + 2, :],
                                start=(k == 0), stop=(k == KH - 2),
                                perf_mode=DR)
                    blk = br * S + s
                    q, r = divmod(blk, 4)
                    nc.scalar.activation(
                        ldst[32 * r:32 * r + nh, q * BL:(q + 1) * BL],
                        hd[0:nh, :], AF.Identity, bias=bh[:], scale=inv)
                    h_prev = h_new

            p12_cm.__exit__(None, None, None)

            # ---------------- phase 3: deferred log-softmax ---------------
            with tc.tile_pool(name="epi", bufs=1) as epi, \
                 tc.tile_pool(name="expp", bufs=4) as expp, \
                 tc.tile_pool(name="lpp", bufs=6) as lpp, \
                 tc.tile_pool(name="eps", bufs=4, space="PSUM") as eps:
                oselt_s = epi.tile([128, NQ * BL], f32)
                nc.sync.dma_start(oselt_s[:], oselt_d[:])
                oselm_s = epi.tile([128, NQ * BL], f32)
                nc.sync.dma_start(oselm_s[:], oselm_d[:])
                redt_s = epi.tile([128, 4], f32)
                nc.sync.dma_start(redt_s[:], redt_d[:])
                redm_s = epi.tile([128, 4], f32)
                nc.sync.dma_start(redm_s[:], redm_d[:])

                for is_m in range(2 if do_epi else 0):
                    lsrc = lm_s if is_m else lt_s
                    osel = oselm_s if is_m else oselt_s
                    red = redm_s if is_m else redt_s
                    out_d = mout_d if is_m else tout_d
                    # dst view: block (b*10+s) = 4q+r -> [r, q, n]
                    dst = out_d[:].rearrange("b s n -> (b s) n").rearrange(
                        "(q r) n -> r q n", r=4)
                    for q in range(NQ):
                        col = slice(q * BL, (q + 1) * BL)
                        ex = expp.tile([128, BL], f32, tag="ex")
                        nc.scalar.activation(ex[:], lsrc[:, col], AF.Exp)
                        pr = expp.tile([128, BL], f32, tag="pr")
                        nc.vector.tensor_mul(pr[:], lsrc[:, col], osel[:, col])
                        se = eps.tile([4, BL], f32, tag="eps")
                        nc.tensor.matmul(se[:], red[:], ex[:],
                                         start=True, stop=True)
                        lnz = lpp.tile([4, BL], f32, tag="lnz")
                        nc.scalar.activation(lnz[:], se[:], AF.Ln)
                        ch = eps.tile([4, BL], f32, tag="eps")
                        nc.tensor.matmul(ch[:], red[:], pr[:],
                                         start=True, stop=True)
                        lp = lpp.tile([4, BL], f32, tag="lp")
                        nc.vector.tensor_sub(lp[:], ch[:], lnz[:])
                        nc.sync.dma_start(dst[:, q, :], lp[:])

    nc.finalize()
    return nc


def _prep_core_inputs(z1, z2, old_transform, old_magnitude, shared, core):
    sl = slice(core * BL, (core + 1) * BL)
    ot = old_transform[sl]   # (BL, 2, S) int32
    om = old_magnitude[sl]

    def ztile(z):
        # (BL, ENC) -> (128, KZ, BL) bf16, [p, k, n] = z[n, k*128+p]
        zt = np.ascontiguousarray(z[sl].T).reshape(KZ, 128, BL)
        return zt.transpose(1, 0, 2)

    zT = np.stack([ztile(z1), ztile(z2)], axis=1)  # (128, 2, KZ, BL)

    # per-call action index -> one-hot, replicated on 4x32-partition strips;
    # layout (128, NCALL, BL) so a single SBUF tile holds all calls
    oneh = np.zeros((128, NCALL, BL), np.float32)
    cols = np.arange(BL)
    prev = np.full(BL, VOCAB - 1, np.int64)  # start token
    li = 0
    for br in range(2):
        for s in range(S):
            for g in range(4):
                oneh[32 * g + prev, li, cols] = 1.0   # transform cell input
            li += 1
            a = ot[:, br, s].astype(np.int64)
            for g in range(4):
                oneh[32 * g + a, li, cols] = 1.0      # magnitude cell input
            li += 1
            prev = om[:, br, s].astype(np.int64)

    # packed head-selection one-hots
    oselt = np.zeros((128, NQ * BL), np.float32)
    oselm = np.zeros((128, NQ * BL), np.float32)
    for br in range(2):
        for s in range(S):
            q, r = divmod(br * S + s, 4)
            oselt[32 * r + ot[:, br, s].astype(np.int64), q * BL + cols] = 1.0
            oselm[32 * r + om[:, br, s].astype(np.int64), q * BL + cols] = 1.0

    m = {
        "zT": zT.astype(ml_dtypes.bfloat16),
        "oneh": oneh.astype(ml_dtypes.bfloat16),
        "oselt": oselt,
        "oselm": oselm,
    }
    m.update(shared)
    return m


def _pad16(W, mm):
    if mm == "bf16":
        return W
    out = np.zeros((16, W.shape[1]), W.dtype)
    out[:W.shape[0]] = W
    return out


def _prep_shared(action_emb, branch_emb, W_ih, W_hh, b_ih, b_hh, Wt, bt, Wm, bm,
                 mm=None):
    mm = MM if mm is None else mm
    sc = 1.0 if mm == "bf16" else WSCALE
    sch = 1.0 if mm == "bf16" else WSCALE / HSCALE  # h carries HSCALE itself
    np8 = ml_dtypes.bfloat16 if mm == "bf16" else ml_dtypes.float8_e4m3
    Wz = W_ih[:, :ENC]
    Wbid = W_ih[:, ENC:ENC + EMB]
    Wact = W_ih[:, ENC + EMB:]
    b_all = (b_ih + b_hh + Wbid @ branch_emb[0]).astype(np.float32)

    def kt(WT, kdim, mdim):
        # (mdim, kdim) weight -> lhsT tiles (128, kdim/128, mdim)
        return np.ascontiguousarray(
            WT.T.reshape(kdim // 128, 128, mdim).transpose(1, 0, 2))

    whh = kt(W_hh * sch, DEC, 4 * DEC)
    if mm == "drsw":
        # per (k-pair, m-tile): columns interleaved (A,B) and reversed
        a = whh[:, 0::2].reshape(128, KP, MT, 128)
        b = whh[:, 1::2].reshape(128, KP, MT, 128)
        iv = np.empty((128, KP, MT, 128, 2), np.float32)
        iv[..., 0] = a[..., ::-1]
        iv[..., 1] = b[..., ::-1]
        whh_in = iv.reshape(128, KP, 2 * 4 * DEC).astype(np8)
    else:
        whh_in = whh.astype(np8)

    redt = np.zeros((128, 4), np.float32)
    redm = np.zeros((128, 4), np.float32)
    for r in range(4):
        redt[32 * r:32 * r + NT, r] = 1.0
        redm[32 * r:32 * r + NM, r] = 1.0

    return {
        "wzT": kt(Wz * sc, ENC, 4 * DEC).astype(ml_dtypes.bfloat16),
        "whhT": whh_in,
        "tact": np.tile((action_emb @ Wact.T) * sc, (8, 1))[:128].astype(
            ml_dtypes.bfloat16),
        "ball": np.ascontiguousarray(b_all.reshape(MT, 128).T),
        "wtT": kt(_pad16(Wt * sch, mm), DEC,
                  NT if mm == "bf16" else 16).astype(np8),
        "wmT": kt(_pad16(Wm * sch, mm), DEC,
                  NM if mm == "bf16" else 16).astype(np8),
        "bt": bt.reshape(NT, 1).astype(np.float32),
        "bm": bm.reshape(NM, 1).astype(np.float32),
        "redt": redt,
        "redm": redm,
    }


def kernel(z1, z2, action_emb, branch_emb, W_ih, W_hh, b_ih, b_hh,
           Wt, bt, Wm, bm, old_transform, old_magnitude,
           _trace=False, _tmpdir=None):
    if "nc" not in _CACHE:
        _CACHE["nc"] = _build_program()
    nc = _CACHE["nc"]

    z1 = np.asarray(z1, np.float32)
    z2 = np.asarray(z2, np.float32)
    shared = _prep_shared(np.asarray(action_emb, np.float32),
                          np.asarray(branch_emb, np.float32),
                          np.asarray(W_ih, np.float32),
                          np.asarray(W_hh, np.float32),
                          np.asarray(b_ih, np.float32),
                          np.asarray(b_hh, np.float32),
                          np.asarray(Wt, np.float32),
                          np.asarray(bt, np.float32),
                          np.asarray(Wm, np.float32),
                          np.asarray(bm, np.float32))
    old_transform = np.asarray(old_transform)
    old_magnitude = np.asarray(old_magnitude)
    in_maps = [
        _prep_core_inputs(z1, z2, old_transform, old_magnitude, shared, c)
        for c in range(NCORES)
    ]

    kw = {}
    if _trace:
        kw = dict(trace=True, tmpdir=_tmpdir)
    out = None
    last_exc = None
    for attempt in range(3):  # transient NRT device errors happen; retry
        try:
            out = run_bass_kernel_spmd(
                nc, in_maps, core_ids=list(range(NCORES)), **kw)
            break
        except Exception as e:
            last_exc = e
            import time as _time
            _time.sleep(5 * (attempt + 1))
    if out is None:
        raise last_exc

    t_lp = np.concatenate(
        [r["tout"].transpose(2, 0, 1) for r in out.results], axis=0)
    m_lp = np.concatenate(
        [r["mout"].transpose(2, 0, 1) for r in out.results], axis=0)
    res = (old_transform, t_lp, old_magnitude, m_lp)
    if _trace:
        return res, out
    return res
